# revision 3
# baseline (speedup 1.0000x reference)
"""Trainium2 Bass kernel: pairwise BiLSTM head/mod scorer (ConcatHeadModule).

Computes scores[i, j] = sum_h v[h] * tanh(A[i,h] + B[j,h]) + outBias where
  A = tanh(x_i @ W_foh + cb_h) @ hid2Layer[:H] + hid2Bias   (i-shard rows)
  B = tanh(x_j @ W_fom + cb_m) @ hid2Layer[H:]              (all j rows)
with n=1024, 2L=512, H=512, H2=256.

Key trick: the O(n^2 * H2) pairwise tanh (the baseline's ACT-engine
bottleneck, ~218us/core) is replaced by an odd-harmonic sine expansion
  tanh(x) ~= sum_k alpha_k sin(k*w0*x),  k in {1,3,5,7,9}
(max err 4.1e-3 on the attained range |A+B| <= 6.2), which splits by angle
addition into rank-2 products per harmonic:
  sin(kw0(A+B)) = sin(kw0 A)cos(kw0 B) + cos(kw0 A)sin(kw0 B)
so scores become 4*5 matmuls of [128i,256h]@[256h,1024j] on the PE array and
the trig feature maps are computed on the small factor matrices only
(~100x less elementwise work than the baseline).

ACT Sin only accepts arguments in [-pi, pi] (no HW range reduction, and the
DVE has no mod op), so harmonics are built from in-range ACT bases plus
bf16 DVE product recurrences:
  s1 = Sin(w0 X), c1 = Sin(w0 X + pi/2), s2 = Sin(2w0 X)   (all in-range)
  c2 = Sin(pi/2 - |2w0 X|), c3 = Sin(pi/2 - |3w0 X|)       (cos via Abs fold)
  s3 = 2*c2*s1 + s1;  x_{k+2} = 2*c2*x_k - x_{k-2} for k+2 in {5,7,9}
Each recurrence map costs one TT mult + one STT on the DVE (bf16, 2x mode).
End-to-end rel err vs fp64 reference: 5.5e-3 (gate 2e-2).

Sharding: head axis i split 8 ways (128 rows/core); weights + full x
replicated per core. Output [128, 1024] per core, concatenated on host.
"""

import numpy as np

N = 1024          # tokens (head and mod axes)
L2 = 512          # 2*L, BiLSTM concat width
H = 512           # hidden (headfov/modfov width)
H2 = 256          # hidden2 width
NCORES = 8
SHARD = N // NCORES   # 128 head rows per core
P = 128

# Odd-harmonic sine fit of tanh on [-6.3, 6.3] (near-minimax, max err 4.06e-3)
W0 = 0.346827
KS = (1, 3, 5, 7, 9)
ALPHAS = (1.21297423, 0.28093357, 0.09147945, 0.03092532, 0.0112292)
NK = len(KS)
HPI = 1.5707964   # fp32 pi/2

_CACHE = {}


def _build_nc():
    """Build + compile the per-core Bass module (SPMD: same NEFF, 8 cores)."""
    from contextlib import ExitStack

    import concourse.mybir as mybir
    import concourse.tile as tile
    from concourse import bacc

    fp32 = mybir.dt.float32
    bf16 = mybir.dt.bfloat16
    AF = mybir.ActivationFunctionType
    ALU = mybir.AluOpType

    nc = bacc.Bacc("TRN2", debug=False, enable_asserts=False, num_devices=NCORES)

    # Inputs pre-arranged on host to the exact SBUF image [128, F]
    # (k-chunks of 128 along partitions, chunk-major on the free dim).
    d_xts = nc.dram_tensor("xts", [P, 4 * SHARD], bf16, kind="ExternalInput").ap()
    d_xtf = nc.dram_tensor("xtf", [P, 4 * N], bf16, kind="ExternalInput").ap()
    d_wfoh = nc.dram_tensor("wfoh", [P, 4 * H], bf16, kind="ExternalInput").ap()
    d_wfom = nc.dram_tensor("wfom", [P, 4 * H], bf16, kind="ExternalInput").ap()
    d_h2a = nc.dram_tensor("h2a", [P, 4 * H2], bf16, kind="ExternalInput").ap()
    d_h2b = nc.dram_tensor("h2b", [P, 4 * H2], bf16, kind="ExternalInput").ap()
    d_cbh = nc.dram_tensor("cbh", [P, 4], fp32, kind="ExternalInput").ap()
    d_cbm = nc.dram_tensor("cbm", [P, 4], fp32, kind="ExternalInput").ap()
    d_h2bias = nc.dram_tensor("h2bias", [P, 2], fp32, kind="ExternalInput").ap()
    d_coef = nc.dram_tensor("coef", [P, 2 * NK], fp32, kind="ExternalInput").ap()
    d_ob = nc.dram_tensor("ob", [P, 1], fp32, kind="ExternalInput").ap()
    d_out = nc.dram_tensor("scores", [SHARD, N], fp32, kind="ExternalOutput").ap()

    with tile.TileContext(nc) as tc, ExitStack() as ctx:
        persist = ctx.enter_context(tc.tile_pool(name="persist", bufs=1))
        BbT = persist.tile([P, 2 * N], fp32)        # (hc, j): B factors
        ApT = persist.tile([P, 2 * SHARD], fp32)    # (hc, i): A factors
        coef_sb = persist.tile([P, 2 * NK], fp32)
        ob_sb = persist.tile([P, 1], fp32)
        hpi_sb = persist.tile([P, 1], fp32)         # +pi/2 bias for ACT
        nc.vector.memset(hpi_sb[:, :], HPI)
        nc.sync.dma_start(coef_sb[:, :], d_coef)
        nc.sync.dma_start(ob_sb[:, :], d_ob)

        # ---------------- preamble: A^T and B^T factor matrices ----------------
        with tc.tile_pool(name="pre", bufs=1) as pre, \
             tc.tile_pool(name="pps", bufs=2, space="PSUM") as pps:
            wfoh_sb = pre.tile([P, 4 * H], bf16)
            wfom_sb = pre.tile([P, 4 * H], bf16)
            h2a_sb = pre.tile([P, 4 * H2], bf16)
            h2b_sb = pre.tile([P, 4 * H2], bf16)
            xts_sb = pre.tile([P, 4 * SHARD], bf16)
            xtf_sb = pre.tile([P, 4 * N], bf16)
            cbh_sb = pre.tile([P, 4], fp32)
            cbm_sb = pre.tile([P, 4], fp32)
            h2bias_sb = pre.tile([P, 2], fp32)
            # DMA order follows the critical path: the am^T/Bb^T chain feeds
            # the b-side trig maps, so its inputs go first.
            for sb, dr in ((xtf_sb, d_xtf), (cbm_sb, d_cbm), (wfom_sb, d_wfom),
                           (h2b_sb, d_h2b), (xts_sb, d_xts), (cbh_sb, d_cbh),
                           (wfoh_sb, d_wfoh), (h2a_sb, d_h2a),
                           (h2bias_sb, d_h2bias)):
                nc.sync.dma_start(sb[:, :], dr)

            # am^T = tanh(W_fom^T @ x^T + cb_m)   [512f x 1024j]
            amT = pre.tile([P, 4 * N], bf16)  # (ft, j)
            for ft in range(4):
                for jh in range(2):
                    ps = pps.tile([P, 512], fp32, tag="ps_b")
                    for kc in range(4):
                        nc.tensor.matmul(
                            ps[:, :],
                            lhsT=wfom_sb[:, kc * H + ft * P: kc * H + (ft + 1) * P],
                            rhs=xtf_sb[:, kc * N + jh * 512: kc * N + (jh + 1) * 512],
                            start=(kc == 0), stop=(kc == 3))
                    nc.scalar.activation(
                        amT[:, ft * N + jh * 512: ft * N + (jh + 1) * 512],
                        ps[:, :], AF.Tanh, bias=cbm_sb[:, ft:ft + 1])

            # ah^T = tanh(W_foh^T @ x_shard^T + cb_h)   [512f x 128i]
            ahT = pre.tile([P, H], bf16)  # (ft, i)
            for ft in range(4):
                ps = pps.tile([P, SHARD], fp32, tag="ps_s")
                for kc in range(4):
                    nc.tensor.matmul(
                        ps[:, :],
                        lhsT=wfoh_sb[:, kc * H + ft * P: kc * H + (ft + 1) * P],
                        rhs=xts_sb[:, kc * SHARD: (kc + 1) * SHARD],
                        start=(kc == 0), stop=(kc == 3))
                nc.scalar.activation(ahT[:, ft * P:(ft + 1) * P], ps[:, :],
                                     AF.Tanh, bias=cbh_sb[:, ft:ft + 1])

            # Bb^T = hid2Layer[H:]^T @ am^T   [256h x 1024j]
            for hc in range(2):
                for jh in range(2):
                    ps = pps.tile([P, 512], fp32, tag="ps_b")
                    for kc in range(4):
                        nc.tensor.matmul(
                            ps[:, :],
                            lhsT=h2b_sb[:, kc * H2 + hc * P: kc * H2 + (hc + 1) * P],
                            rhs=amT[:, kc * N + jh * 512: kc * N + (jh + 1) * 512],
                            start=(kc == 0), stop=(kc == 3))
                    nc.vector.tensor_copy(
                        BbT[:, hc * N + jh * 512: hc * N + (jh + 1) * 512], ps[:, :])

            # A^T = hid2Layer[:H]^T @ ah^T + hid2Bias   [256h x 128i]
            for hc in range(2):
                ps = pps.tile([P, SHARD], fp32, tag="ps_s")
                for kc in range(4):
                    nc.tensor.matmul(
                        ps[:, :],
                        lhsT=h2a_sb[:, kc * H2 + hc * P: kc * H2 + (hc + 1) * P],
                        rhs=ahT[:, kc * P:(kc + 1) * P],
                        start=(kc == 0), stop=(kc == 3))
                nc.scalar.activation(ApT[:, hc * SHARD:(hc + 1) * SHARD], ps[:, :],
                                     AF.Identity, bias=h2bias_sb[:, hc:hc + 1])

        w1 = float(np.float32(W0))
        w2 = float(np.float32(2 * W0))
        w3 = float(np.float32(3 * W0))

        def emit_bases(X, F, pool, zpool, label):
            """ACT trig bases of X [P, F]: s1,c1,s2 direct; c2,c3 via Abs fold.
            Emission order puts the recurrence-critical maps (s1, c2) first."""
            m = {}
            m["s1"] = pool.tile([P, F], bf16, name=f"{label}s1")
            nc.scalar.activation(m["s1"][:, :], X[:, :], AF.Sin, scale=w1)
            z2 = zpool.tile([P, F], fp32, tag="z")
            nc.scalar.activation(z2[:, :], X[:, :], AF.Abs, scale=w2)
            m["c2"] = pool.tile([P, F], bf16, name=f"{label}c2")
            nc.scalar.activation(m["c2"][:, :], z2[:, :], AF.Sin, scale=-1.0,
                                 bias=hpi_sb[:, 0:1])
            m["c1"] = pool.tile([P, F], bf16, name=f"{label}c1")
            nc.scalar.activation(m["c1"][:, :], X[:, :], AF.Sin, scale=w1,
                                 bias=hpi_sb[:, 0:1])
            m["s2"] = pool.tile([P, F], bf16, name=f"{label}s2")
            nc.scalar.activation(m["s2"][:, :], X[:, :], AF.Sin, scale=w2)
            z3 = zpool.tile([P, F], fp32, tag="z")
            nc.scalar.activation(z3[:, :], X[:, :], AF.Abs, scale=w3)
            m["c3"] = pool.tile([P, F], bf16, name=f"{label}c3")
            nc.scalar.activation(m["c3"][:, :], z3[:, :], AF.Sin, scale=-1.0,
                                 bias=hpi_sb[:, 0:1])
            return m

        def emit_ladder_step(m, k, F, pool, tpool, label):
            """DVE recurrence x_{k} = 2*c2*x_{k-2} - x_{k-4} (k in 5,7,9);
            s3 = 2*c2*s1 + s1. Two DVE ops per map (TT mult + STT)."""
            sk, ck = f"s{k}", f"c{k}"
            if k == 3:
                t = tpool.tile([P, F], bf16, tag="lt")
                nc.vector.tensor_tensor(t[:, :], m["c2"][:, :], m["s1"][:, :],
                                        ALU.mult)
                m["s3"] = pool.tile([P, F], bf16, name=f"{label}s3")
                nc.vector.scalar_tensor_tensor(m["s3"][:, :], t[:, :], 2.0,
                                               m["s1"][:, :], ALU.mult, ALU.add)
            else:
                sp, cp = f"s{k - 2}", f"c{k - 2}"
                spp, cpp = f"s{k - 4}", f"c{k - 4}"
                t = tpool.tile([P, F], bf16, tag="lt")
                nc.vector.tensor_tensor(t[:, :], m["c2"][:, :], m[sp][:, :],
                                        ALU.mult)
                m[sk] = pool.tile([P, F], bf16, name=f"{label}{sk}")
                nc.vector.scalar_tensor_tensor(m[sk][:, :], t[:, :], 2.0,
                                               m[spp][:, :], ALU.mult,
                                               ALU.subtract)
                t2 = tpool.tile([P, F], bf16, tag="lt")
                nc.vector.tensor_tensor(t2[:, :], m["c2"][:, :], m[cp][:, :],
                                        ALU.mult)
                m[ck] = pool.tile([P, F], bf16, name=f"{label}{ck}")
                nc.vector.scalar_tensor_tensor(m[ck][:, :], t2[:, :], 2.0,
                                               m[cpp][:, :], ALU.mult,
                                               ALU.subtract)

        # ---------------- a-side maps (small: [256h x 128i]) ----------------
        amap_pool = ctx.enter_context(tc.tile_pool(name="amaps", bufs=1))
        az_pool = ctx.enter_context(tc.tile_pool(name="az", bufs=2))
        am = emit_bases(ApT, 2 * SHARD, amap_pool, az_pool, "a")
        for k in (3, 5, 7, 9):
            emit_ladder_step(am, k, 2 * SHARD, amap_pool, az_pool, "a")
        # fold alpha_k * v[h] into bf16 lhsT maps
        afold_pool = ctx.enter_context(tc.tile_pool(name="afold", bufs=1))
        a_folded = []
        for idx, k in enumerate(KS):
            saF = afold_pool.tile([P, 2 * SHARD], bf16, name=f"saF{k}")
            caF = afold_pool.tile([P, 2 * SHARD], bf16, name=f"caF{k}")
            for hc in range(2):
                cs = coef_sb[:, idx * 2 + hc: idx * 2 + hc + 1]
                nc.vector.tensor_scalar_mul(
                    saF[:, hc * P:(hc + 1) * P],
                    am[f"s{k}"][:, hc * P:(hc + 1) * P], cs)
                nc.vector.tensor_scalar_mul(
                    caF[:, hc * P:(hc + 1) * P],
                    am[f"c{k}"][:, hc * P:(hc + 1) * P], cs)
            a_folded.append((saF, caF))

        # ---------------- b-side maps + rank-2 matmuls ----------------
        bmap_pool = ctx.enter_context(tc.tile_pool(name="bmaps", bufs=1))
        bz_pool = ctx.enter_context(tc.tile_pool(name="bz", bufs=2))
        mps = ctx.enter_context(tc.tile_pool(name="mps", bufs=1, space="PSUM"))
        ps_acc = [mps.tile([P, 512], fp32, name=f"acc{jh}") for jh in range(2)]
        spool = ctx.enter_context(tc.tile_pool(name="stg", bufs=1))

        bm = emit_bases(BbT, 2 * N, bmap_pool, bz_pool, "b")

        def emit_mms(idx, k):
            saF, caF = a_folded[idx]
            first, last = (idx == 0), (idx == NK - 1)
            # scores += saF^T @ c_k(B) + caF^T @ s_k(B)
            for ti, (aF, bmap) in enumerate(((saF, bm[f"c{k}"]),
                                             (caF, bm[f"s{k}"]))):
                for hc in range(2):
                    for jh in range(2):
                        nc.tensor.matmul(
                            ps_acc[jh][:, :],
                            lhsT=aF[:, hc * P:(hc + 1) * P],
                            rhs=bmap[:, hc * N + jh * 512: hc * N + (jh + 1) * 512],
                            start=(first and ti == 0 and hc == 0),
                            stop=(last and ti == 1 and hc == 1),
                            skip_group_check=True)

        emit_mms(0, 1)
        for idx, k in enumerate((3, 5, 7, 9), start=1):
            emit_ladder_step(bm, k, 2 * N, bmap_pool, bz_pool, "b")
            emit_mms(idx, k)

        # psum accumulators (+outBias) -> staging -> one contiguous DMA out
        stg = spool.tile([P, N], fp32)
        for jh in range(2):
            nc.vector.tensor_scalar_add(stg[:, jh * 512:(jh + 1) * 512],
                                        ps_acc[jh][:, :], ob_sb[:, 0:1])
        nc.sync.dma_start(d_out[:, :], stg[:, :])

    nc.compile()
    return nc


def get_nc():
    if "nc" not in _CACHE:
        _CACHE["nc"] = _build_nc()
    return _CACHE["nc"]


def _chunk_p(a, dtype=np.float32):
    """[c*128, M] -> SBUF image [128, c*M] (chunk-major free dim)."""
    k, m = a.shape
    c = k // P
    return np.ascontiguousarray(
        a.reshape(c, P, m).transpose(1, 0, 2).reshape(P, c * m), dtype=dtype)


def make_in_maps(inputs):
    lstms0 = np.asarray(inputs["lstms0"], dtype=np.float32)
    lstms1 = np.asarray(inputs["lstms1"], dtype=np.float32)
    w_foh = np.asarray(inputs["W_foh"], dtype=np.float32)
    w_fom = np.asarray(inputs["W_fom"], dtype=np.float32)
    cat_bias = np.asarray(inputs["catBias"], dtype=np.float32)
    hid2 = np.asarray(inputs["hid2Layer"], dtype=np.float32)
    hid2_bias = np.asarray(inputs["hid2Bias"], dtype=np.float32)
    out_layer = np.asarray(inputs["outLayer"], dtype=np.float32)
    out_bias = np.asarray(inputs["outBias"], dtype=np.float32)

    import ml_dtypes

    bf16 = ml_dtypes.bfloat16
    x = np.concatenate([lstms0, lstms1], axis=1)          # [1024, 512]
    xtf = _chunk_p(np.ascontiguousarray(x.T), bf16)       # [128, 4096]
    wfoh = _chunk_p(w_foh, bf16)
    wfom = _chunk_p(w_fom, bf16)
    h2a = _chunk_p(hid2[:H], bf16)
    h2b = _chunk_p(hid2[H:], bf16)
    cbh = np.ascontiguousarray(cat_bias[0, :H].reshape(4, P).T, dtype=np.float32)
    cbm = np.ascontiguousarray(cat_bias[0, H:].reshape(4, P).T, dtype=np.float32)
    h2bias = np.ascontiguousarray(hid2_bias[0].reshape(2, P).T, dtype=np.float32)
    # coef[:, idx*2+hc] = alpha_k * v[hc*128 + p]
    coef = np.empty((P, 2 * NK), dtype=np.float32)
    for idx in range(NK):
        for hc in range(2):
            coef[:, idx * 2 + hc] = ALPHAS[idx] * out_layer[hc * P:(hc + 1) * P, 0]
    ob = np.full((P, 1), float(out_bias[0, 0]), dtype=np.float32)

    in_maps = []
    for c in range(NCORES):
        xts = _chunk_p(np.ascontiguousarray(x[c * SHARD:(c + 1) * SHARD].T), bf16)
        in_maps.append(dict(xts=xts, xtf=xtf, wfoh=wfoh, wfom=wfom, h2a=h2a,
                            h2b=h2b, cbh=cbh, cbm=cbm, h2bias=h2bias,
                            coef=coef, ob=ob))
    return in_maps


def kernel(**inputs):
    from concourse.bass_utils import run_bass_kernel_spmd

    nc = get_nc()
    in_maps = make_in_maps(inputs)
    res = run_bass_kernel_spmd(nc, in_maps, core_ids=list(range(NCORES)))
    out = np.concatenate([res.results[c]["scores"] for c in range(NCORES)], axis=0)
    return np.ascontiguousarray(out, dtype=np.float32)
